# revision 32
# baseline (speedup 1.0000x reference)
"""Trainium2 Bass kernel for nn_MultiHeadAttention (B=4, T=2048, D=1024,
H=16, d_k=64) on 8 NeuronCores.

Sharding: tensor-parallel over heads — core c computes heads {2c, 2c+1} for
ALL batches (W_q/W_k/W_v column-sharded, W_o row-sharded). The final
all-reduce of the output projection is replaced by a host-side sum of the 8
partial outputs. Per-batch attention length (ceil(valid_len/128) Tk tiles)
is baked into the single SPMD program, keeping every core's instruction
stream identical AND load-balanced (each core owns 2 heads of every batch).

Key device tricks:
  - scores^T layout (Tk on partitions, Tq on free): QK^T matmuls are
    row-tile-paired (two K=64 head matmuls share the 128x128 PE array via
    tile_position), the padding mask is applied through the exp activation's
    per-partition bias operand (0 or -30000), and the softmax denominator is
    produced by a ones-column folded into the P@V matmul (lhsT = [V_h | 1]).
  - 1/den is broadcast across partitions with a K=1 matmul.
  - trn2 encodes at most one semaphore wait per instruction; a post-pass
    splits any multi-wait instruction Tile emits into single-wait
    InstEventSemaphore ops (walrus rejects them otherwise).
"""
import os
import sys

for _p in ("/opt/trn_rl_repo", "/root/.axon_site/_ro/trn_rl_repo"):
    if os.path.isdir(_p) and _p not in sys.path:
        sys.path.append(_p)

import numpy as np
import ml_dtypes

import concourse.bass as bass
import concourse.mybir as mybir
import concourse.tile as tile
from concourse.bass import ts
from concourse.bass_utils import run_bass_kernel_spmd

D = 1024
T = 2048
H = 16
DK = 64
P = 128
KC = D // P          # 8 contraction chunks for the projections
NT = T // 512        # 4 Tq chunks of 512
TC = T // P          # 16 Tk tiles / T chunks
NCORES = 8
MASK_NEG = -30000.0

F32 = mybir.dt.float32
F32R = mybir.dt.float32r
BF16 = mybir.dt.bfloat16
AF = mybir.ActivationFunctionType
BF16_NP = ml_dtypes.bfloat16


def _split_multi_waits(nc):
    """trn2 instructions encode at most one sync wait; split the rest into
    standalone single-wait event-semaphore ops (see module docstring)."""
    n_split = 0
    for f in nc.m.functions:
        for blk in f.blocks:
            insts = blk.instructions
            out = []
            changed = False
            for inst in insts:
                si = inst.sync_info
                if si is not None and len(si.on_wait) > 1:
                    waits = list(si.on_wait)
                    for k, wt in enumerate(waits[:-1]):
                        ev = mybir.InstEventSemaphore(
                            name=f"{inst.name}_wsplit{k}",
                            engine=inst.engine,
                            ins=[],
                            outs=[],
                            bass_nofuse=True,
                            sync_info=mybir.SyncInfo(on_wait=[wt], on_update=[]),
                        )
                        out.append(ev)
                        n_split += 1
                    inst.sync_info = mybir.SyncInfo(
                        on_wait=[waits[-1]], on_update=si.on_update
                    )
                    changed = True
                out.append(inst)
            if changed:
                blk.instructions = out
    return n_split


def build_nc(NB, CPB, J_list, dt_x):
    """Build the SPMD program.

    NB     : number of batch slots handled per core
    CPB    : projection output columns per core per batch (n_heads_per_core*64)
    J_list : per batch slot, number of 128-row Tk tiles of attention
    dt_x   : dtype of x/weights/intermediates (BF16 or F32R)
    """
    NPAIR = CPB // P  # head pairs per batch slot
    nc = bass.Bass()

    # window-major layout: one [P, KC, 512] window is contiguous per
    # partition (8 KB runs) so each DMA needs only 128 descriptors
    xq_d = [nc.declare_dram_parameter(f"xq{s}", [NT, P, KC, 512], dt_x,
                                      isOutput=False) for s in range(NB)]
    xk_d = [nc.declare_dram_parameter(f"xk{s}", [NT, P, KC, 512], dt_x,
                                      isOutput=False) for s in range(NB)]
    xv_d = [nc.declare_dram_parameter(f"xv{s}", [NT, P, KC, 512], dt_x,
                                      isOutput=False) for s in range(NB)]
    wq_d = nc.declare_dram_parameter("wq", [P, KC, CPB], dt_x, isOutput=False)
    wk_d = nc.declare_dram_parameter("wk", [P, KC, CPB], dt_x, isOutput=False)
    wv_d = nc.declare_dram_parameter("wv", [P, KC, CPB], dt_x, isOutput=False)
    wo_d = nc.declare_dram_parameter("wo", [P, NPAIR, D], dt_x, isOutput=False)
    bq_d = nc.declare_dram_parameter("bq", [P, NPAIR], F32, isOutput=False)
    bk_d = nc.declare_dram_parameter("bk", [P, NPAIR], F32, isOutput=False)
    bv_d = nc.declare_dram_parameter("bv", [1, CPB], dt_x, isOutput=False)
    mb_d = [nc.declare_dram_parameter(f"mb{s}", [P, TC], F32, isOutput=False)
            for s in range(NB)]
    onesb_d = nc.declare_dram_parameter("onesb", [1, DK], F32R, isOutput=False)
    o_d = [nc.declare_dram_parameter(f"o{s}", [T, D], BF16, isOutput=True)
           for s in range(NB)]

    with tile.TileContext(nc) as tc:
        with (
            tc.tile_pool(name="pers", bufs=1) as pers,
            tc.tile_pool(name="stream", bufs=2) as stream,
            tc.tile_pool(name="attn", bufs=5) as attn_pool,
            tc.tile_pool(name="small", bufs=3) as small,
            tc.tile_pool(name="outp", bufs=4) as outp,
            tc.tile_pool(name="ps_proj", bufs=2, space="PSUM") as ps_proj,
            tc.tile_pool(name="ps_qk", bufs=3, space="PSUM") as ps_qk,
            tc.tile_pool(name="ps_pv", bufs=2, space="PSUM") as ps_pv,
            tc.tile_pool(name="ps_tr", bufs=1, space="PSUM") as ps_tr,
        ):
            # ---- persistent tensors -------------------------------------
            wq = pers.tile([P, KC, CPB], dt_x, name="wq")
            wk = pers.tile([P, KC, CPB], dt_x, name="wk")
            wv = pers.tile([P, KC, CPB], dt_x, name="wv")
            wo = pers.tile([P, NPAIR, D], dt_x, name="wo")
            bq = pers.tile([P, NPAIR], F32, name="bq")
            bk = pers.tile([P, NPAIR], F32, name="bk")
            bv = pers.tile([1, CPB], dt_x, name="bv")
            nc.sync.dma_start(wq[:], wq_d[:])
            nc.sync.dma_start(wk[:], wk_d[:])
            nc.sync.dma_start(wv[:], wv_d[:])
            nc.sync.dma_start(wo[:], wo_d[:])
            nc.sync.dma_start(bq[:], bq_d[:])
            nc.sync.dma_start(bk[:], bk_d[:])
            nc.sync.dma_start(bv[:], bv_d[:])
            mb = []
            for s in range(NB):
                t = pers.tile([P, TC], F32, name=f"mb{s}")
                nc.sync.dma_start(t[:], mb_d[s][:])
                mb.append(t)

            ones_t = pers.tile([1, P], dt_x, name="ones_t")   # V-bias fold lhsT
            nc.vector.memset(ones_t[:], 1.0)
            ones_b = pers.tile([1, DK], F32R, name="ones_b")  # 1/den bcast lhsT
            nc.sync.dma_start(ones_b[:], onesb_d[:])

            QT = [pers.tile([P, NPAIR, T], dt_x, name=f"QT{s}") for s in range(NB)]
            KT = [pers.tile([P, NPAIR, T], dt_x, name=f"KT{s}") for s in range(NB)]
            # V with a ones column folded in at free index 64 of each head
            V = [pers.tile([P, TC, 2 * NPAIR, DK + 1], dt_x, name=f"V{s}")
                 for s in range(NB)]

            for s in range(NB):
                nc.vector.memset(V[s][:, :, :, DK], 1.0)

            AO = [pers.tile([P, NPAIR, T], dt_x, name=f"AO{s}")
                  for s in range(NB)]
            NR = 2 * NPAIR * NT  # unnormalized-output rows per slot
            uo = [pers.tile([DK, NR, 512], BF16, name=f"uo{s}")
                  for s in range(NB)]
            dens = [pers.tile([NR, 512], F32, name=f"dens{s}")
                    for s in range(NB)]
            recs = [pers.tile([NR, 512], F32R, name=f"rec{s}")
                    for s in range(NB)]

            def proj_phase(s):
                # ---- projections (contract D on partitions) -------------
                for n in range(NT):
                    xq_w = stream.tile([P, KC, 512], dt_x, tag="xq_w")
                    xk_w = stream.tile([P, KC, 512], dt_x, tag="xk_w")
                    xv_w = stream.tile([P, KC, 512], dt_x, tag="xv_w")
                    nc.sync.dma_start(xq_w[:], xq_d[s][n])
                    nc.sync.dma_start(xk_w[:], xk_d[s][n])
                    nc.sync.dma_start(xv_w[:], xv_d[s][n])
                    # Q^T, K^T tiles: [CPB rows, 512 t]
                    for mc in range(NPAIR):
                        ps_q = ps_proj.tile([P, 512], F32, tag="proj")
                        for kc in range(KC):
                            nc.tensor.matmul(ps_q[:], wq[:, kc, ts(mc, P)],
                                             xq_w[:, kc, :],
                                             start=(kc == 0), stop=(kc == KC - 1))
                        nc.vector.tensor_scalar_add(QT[s][:, mc, ts(n, 512)],
                                                    ps_q[:], bq[:, mc:mc + 1])
                        ps_k = ps_proj.tile([P, 512], F32, tag="proj")
                        for kc in range(KC):
                            nc.tensor.matmul(ps_k[:], wk[:, kc, ts(mc, P)],
                                             xk_w[:, kc, :],
                                             start=(kc == 0), stop=(kc == KC - 1))
                        nc.vector.tensor_scalar_add(KT[s][:, mc, ts(n, 512)],
                                                    ps_k[:], bk[:, mc:mc + 1])
                    # V tiles: [t chunk on partitions, CPB cols]
                    for tc_i in range(4):
                        m = n * 4 + tc_i
                        ps_v_full = ps_proj.tile([P, 512], F32, tag="proj")
                        ps_v = ps_v_full[:, 0:CPB]
                        for kc in range(KC):
                            nc.tensor.matmul(ps_v[:], xv_w[:, kc, ts(tc_i, P)],
                                             wv[:, kc, :],
                                             start=(kc == 0), stop=False)
                        nc.tensor.matmul(ps_v[:], ones_t[0:1, :], bv[0:1, :],
                                         start=False, stop=True)
                        nc.vector.tensor_copy(
                            V[s][:, m, :, 0:DK],
                            ps_v[:].rearrange("p (h d) -> p h d", d=DK))

            def attn_phase(s):
                # Unnormalized head outputs (and the folded den row) are
                # staged to SBUF; normalization is deferred so the DVE
                # reciprocal never stalls the in-order PE stream.
                J = J_list[s]
                for pc in range(NPAIR):
                    for tq in range(NT):
                        ps_o0 = ps_pv.tile([P, 512], F32, tag="pv")
                        ps_o1 = ps_pv.tile([P, 512], F32, tag="pv")
                        ps_os = (ps_o0, ps_o1)
                        for j in range(J):
                            ps_s0 = ps_qk.tile([P, 512], F32, tag="qk")
                            ps_s1 = ps_qk.tile([P, 512], F32, tag="qk")
                            nc.tensor.matmul(ps_s0[:],
                                             KT[s][0:DK, pc, ts(j, P)],
                                             QT[s][0:DK, pc, ts(tq, 512)],
                                             start=True, stop=True,
                                             tile_position=(0, 0))
                            nc.tensor.matmul(ps_s1[:],
                                             KT[s][DK:P, pc, ts(j, P)],
                                             QT[s][DK:P, pc, ts(tq, 512)],
                                             start=True, stop=True,
                                             tile_position=(DK, 0))
                            at = attn_pool.tile([P, 2, 512], dt_x, tag="at")
                            nc.scalar.activation(at[:, 0, :], ps_s0[:], AF.Exp,
                                                 bias=mb[s][:, j:j + 1],
                                                 scale=0.125)
                            nc.scalar.activation(at[:, 1, :], ps_s1[:], AF.Exp,
                                                 bias=mb[s][:, j:j + 1],
                                                 scale=0.125)
                            for h01 in range(2):
                                nc.tensor.matmul(ps_os[h01][0:DK + 1, :],
                                                 V[s][:, j, 2 * pc + h01, :],
                                                 at[:, h01, :],
                                                 start=(j == 0),
                                                 stop=(j == J - 1))
                        for h01 in range(2):
                            r = (pc * NT + tq) * 2 + h01
                            den_st = small.tile([1, 512], F32, tag="den_st")
                            if h01 == 0:
                                nc.scalar.activation(uo[s][:, r, :],
                                                     ps_os[h01][0:DK, :],
                                                     AF.Identity)
                                nc.scalar.activation(den_st[:],
                                                     ps_os[h01][DK:DK + 1, :],
                                                     AF.Identity)
                            else:
                                nc.vector.tensor_copy(uo[s][:, r, :],
                                                      ps_os[h01][0:DK, :])
                                nc.vector.tensor_copy(den_st[:],
                                                      ps_os[h01][DK:DK + 1, :])
                            nc.sync.dma_start(dens[s][r:r + 1, :], den_st[:])

            def norm_outproj_phase(s, last):
                # batched normalization (off the PE critical path)
                with nc.allow_low_precision(
                        reason="f32r output is bit-identical to f32"):
                    nc.vector.reciprocal(recs[s][:], dens[s][:])
                for pc in range(NPAIR):
                    for tq in range(NT):
                        for h01 in range(2):
                            r = (pc * NT + tq) * 2 + h01
                            # stage rec row at partition 0 for the K=1 matmul
                            rst = small.tile([1, 512], F32R, tag="rst")
                            nc.sync.dma_start(rst[:], recs[s][r:r + 1, :])
                            ps_b = ps_tr.tile([DK, 512], F32, tag="bc")
                            nc.tensor.matmul(ps_b[:], ones_b[0:1, :],
                                             rst[0:1, :],
                                             start=True, stop=True)
                            nc.vector.tensor_mul(
                                out=AO[s][ts(h01, DK), pc, ts(tq, 512)],
                                in0=ps_b[:], in1=uo[s][:, r, :])

                # ---- output projection (partial; host sums cores) -------
                for m in range(TC):
                    ot = outp.tile([P, D], BF16, tag="ot")
                    for n2 in range(2):
                        ps_op = ps_proj.tile([P, 512], F32, tag="proj")
                        for cc in range(NPAIR):
                            nc.tensor.matmul(ps_op[:], AO[s][:, cc, ts(m, P)],
                                             wo[:, cc, ts(n2, 512)],
                                             start=(cc == 0),
                                             stop=(cc == NPAIR - 1))
                        if last or (m + n2) % 2 == 0:
                            nc.scalar.activation(ot[:, ts(n2, 512)], ps_op[:],
                                                 AF.Identity)
                        else:
                            nc.vector.tensor_copy(ot[:, ts(n2, 512)], ps_op[:])
                    nc.sync.dma_start(o_d[s][ts(m, P), :], ot[:])

            # Software-pipelined emission: normalization + out-projection of
            # slot s-1 is emitted between attention phases so its DVE/ACT/PE
            # work overlaps the (ACT-paced) attention of later slots.
            proj_phase(0)
            if NB > 1:
                proj_phase(1)
            for s in range(NB):
                attn_phase(s)
                if s + 2 < NB:
                    proj_phase(s + 2)
                if s >= 1:
                    norm_outproj_phase(s - 1, last=False)
            norm_outproj_phase(NB - 1, last=True)

    _split_multi_waits(nc)
    return nc


_CACHE = {}


def _get_nc(NB, CPB, J_list, dt_x):
    key = (NB, CPB, tuple(J_list), str(dt_x))
    if key not in _CACHE:
        _CACHE[key] = build_nc(NB, CPB, J_list, dt_x)
    return _CACHE[key]


def _xt(x, dt_np):
    """[T, D] -> [NT, P, KC, 512] transposed window-major layout."""
    xt = x.T.reshape(KC, P, NT, 512).transpose(2, 1, 0, 3)
    return np.ascontiguousarray(xt).astype(dt_np)


def kernel(**inputs):
    query = np.asarray(inputs["query"], dtype=np.float32)
    key = np.asarray(inputs["key"], dtype=np.float32)
    value = np.asarray(inputs["value"], dtype=np.float32)
    vl = np.asarray(inputs["valid_length"]).astype(np.int64)
    W_q = np.asarray(inputs["W_q"], dtype=np.float32)
    b_q = np.asarray(inputs["b_q"], dtype=np.float32)
    W_k = np.asarray(inputs["W_k"], dtype=np.float32)
    b_k = np.asarray(inputs["b_k"], dtype=np.float32)
    W_v = np.asarray(inputs["W_v"], dtype=np.float32)
    b_v = np.asarray(inputs["b_v"], dtype=np.float32)
    W_o = np.asarray(inputs["W_o"], dtype=np.float32)
    b_o = np.asarray(inputs["b_o"], dtype=np.float32)

    B = query.shape[0]
    NB = B
    CPB = (H // NCORES) * DK       # 2 heads per core -> 128 cols
    NPAIR = CPB // P               # 1
    dt_x = BF16
    dt_np = BF16_NP

    # slot s handles batch order[s]; J (Tk tiles) baked per slot
    order = np.argsort(-vl, kind="stable")
    J_list = []
    for s in range(NB):
        v = int(vl[order[s]])
        J_list.append(TC if v == 0 else max(1, -(-v // P)))

    nc = _get_nc(NB, CPB, J_list, dt_x)

    # host-side shard prep
    xq_np, xk_np, xv_np, mb_np = [], [], [], []
    for s in range(NB):
        b = int(order[s])
        v = int(vl[b])
        q_b = query[b] if v != 0 else np.zeros_like(query[b])
        xq_np.append(_xt(q_b, dt_np))
        xk_np.append(_xt(key[b], dt_np))
        xv_np.append(_xt(value[b], dt_np))
        rows = np.arange(P)[:, None] + P * np.arange(TC)[None, :]
        if v == 0:
            m = np.zeros((P, TC), np.float32)
        else:
            m = np.where(rows < v, 0.0, MASK_NEG).astype(np.float32)
        mb_np.append(m)

    in_maps = []
    for c in range(NCORES):
        c0 = c * CPB
        cols = slice(c0, c0 + CPB)
        im = {
            "wq": np.ascontiguousarray(
                W_q.reshape(KC, P, H * DK).transpose(1, 0, 2)[:, :, cols]
            ).astype(dt_np),
            "wk": np.ascontiguousarray(
                W_k.reshape(KC, P, H * DK).transpose(1, 0, 2)[:, :, cols]
            ).astype(dt_np),
            "wv": np.ascontiguousarray(
                W_v.reshape(KC, P, H * DK).transpose(1, 0, 2)[:, :, cols]
            ).astype(dt_np),
            "wo": np.ascontiguousarray(
                W_o[cols].reshape(NPAIR, P, D).transpose(1, 0, 2)
            ).astype(dt_np),
            "bq": np.ascontiguousarray(
                b_q[cols].reshape(NPAIR, P).T).astype(np.float32),
            "bk": np.ascontiguousarray(
                b_k[cols].reshape(NPAIR, P).T).astype(np.float32),
            "bv": np.ascontiguousarray(b_v[cols][None, :]).astype(dt_np),
            "onesb": np.ones((1, DK), np.float32),
        }
        for s in range(NB):
            im[f"xq{s}"] = xq_np[s]
            im[f"xk{s}"] = xk_np[s]
            im[f"xv{s}"] = xv_np[s]
            im[f"mb{s}"] = mb_np[s]
        in_maps.append(im)

    res = run_bass_kernel_spmd(nc, in_maps, list(range(NCORES)))

    out = np.zeros((B, T, D), np.float32)
    for s in range(NB):
        b = int(order[s])
        acc = np.zeros((T, D), np.float32)
        for c in range(NCORES):
            acc += np.asarray(res.results[c][f"o{s}"]).astype(np.float32)
        out[b] = acc + b_o[None, :]
    return out
